# revision 1
# baseline (speedup 1.0000x reference)
"""Cross multi-head attention kernel for 8 Trainium2 NeuronCores.

Reference computation (per batch b):
    Q = x @ Wq.T ; K = ctx @ Wk.T ; V = ctx @ Wv.T          (16 heads, depth 64)
    scores = (Q_h @ K_h.T) / 8 ; masked where pad_mask -> -inf
    att = softmax(scores) ; out_h = att @ V_h
    y = concat_h(out_h) @ fc_w.T + fc_b
Sharding: 8 cores = 2 batches x 4 head-groups (4 heads each).  Each core
computes a full [E, LQ] bf16 partial of y^T for its batch; the host sums the 4
head-group partials per batch (fp32) and adds the bias.

On-chip layout is fully transposed ("layout B") so no transposes are needed:
    x^T [E, LQ], ctx^T [E, LKV]  ->  Q^T [D,LQ], K^T [D,LKV] per head, V
    natural [LKV, D] augmented with a ones column (att@V emits softmax row
    sums for free on row 64 of the [65, LQ] accumulator).
    scores^T [LKV, LQ] = K^T.T @ Q^T       (contraction over D=64)
    att^T = exp(scores^T * 0.125) * keep_mask^T   (exact-zero masking; no
        row-max needed: scores/8 ~ N(0,1), exp never overflows)
    recip(rowsum) via ln+exp on ACT, broadcast over partitions with a
    zero-padded outer-product matmul, one fused multiply on PSUM evacuation.
    y^T partial [E, LQ] = fcw_part^T.T @ attn^T   (contraction over 256)

Everything runs in bf16 (fp32 accumulation in PSUM).

Schedule notes (from perfetto analysis of previous iterations):
  * DMA issue costs ~0.6us PER dma_start on the issuing engine's queue, so
    input DMAs are spread over four engines (sync:x, gpsimd:wq/mask/fcw,
    scalar:wk+ctx, vector:ctx+wv) to cut kernel startup from 15us to ~8us.
  * The ACT exp over the score tiles (1.1us per [128,1024] tile, 64 tiles)
    is the throughput floor of the attention phase at full PE clock.  Pass 0's
    16 exps are hidden under the V projection by interleaving V tiles with
    pass-0 score matmuls (PSUM: Q/K 2 banks + V 2 + scores 4 = 8), after
    which pass 0's att@V runs as a solid tensor burst.
  * Softmax normalization of pass i is emitted inside pass i+1's kt loop so
    its ACT->matmul chain never stalls the in-order tensor queue (was a
    3.7us bubble per pass).  Pass 3's accumulator is evacuated to SBUF and
    normalized under phase C's kf0 prologue.
  * Phase C: 16 [128,512] chunks; kf0 matmuls run 4 chunks ahead, PSUM
    evacuation alternates ACT/DVE, output DMAs alternate sync/gpsimd.
"""

import os
import sys

import numpy as np

for _p in ("/opt/trn_rl_repo", "/root/.axon_site/_ro/trn_rl_repo"):
    if os.path.isdir(_p) and _p not in sys.path:
        sys.path.insert(0, _p)

import ml_dtypes  # noqa: E402

import concourse.bass as bass  # noqa: E402
import concourse.mybir as mybir  # noqa: E402
import concourse.tile as tile  # noqa: E402
from concourse import bacc  # noqa: E402
from concourse.bass_utils import run_bass_kernel_spmd  # noqa: E402

B, LQ, LKV, E = 2, 1024, 2048, 1024
H_TOTAL, D = 16, 64
NCORES = 8
HGROUPS = 4          # head groups (cores per batch)
HLOCAL = 4           # heads per core
FP = HLOCAL * D      # 256 local head features
P = 128
F32 = mybir.dt.float32
BF16 = mybir.dt.bfloat16
ET = E // P          # 8 contraction tiles for the projections
KT = LKV // P        # 16 key tiles
NQ = LQ // 512       # 2 matmul free-dim chunks
PIPE = 2             # scores run this many kt tiles ahead of att@V


def build_nc() -> bass.Bass:
    nc = bacc.Bacc("TRN2", target_bir_lowering=False)

    xT = nc.dram_tensor("xT", [E, LQ], BF16, kind="ExternalInput")
    ctxT = nc.dram_tensor("ctxT", [E, LKV], BF16, kind="ExternalInput")
    maskT = nc.dram_tensor("maskT", [LKV, LQ], BF16, kind="ExternalInput")
    wqT = nc.dram_tensor("wqT", [E, FP], BF16, kind="ExternalInput")
    wkT = nc.dram_tensor("wkT", [E, FP], BF16, kind="ExternalInput")
    wvT = nc.dram_tensor("wvT", [E, FP], BF16, kind="ExternalInput")
    fcwT = nc.dram_tensor("fcwT", [FP, E], BF16, kind="ExternalInput")
    yT = nc.dram_tensor("yT", [E, LQ], BF16, kind="ExternalOutput")

    with tile.TileContext(nc) as tc:
        with tc.tile_pool(name="persist", bufs=1) as persist:
            QT = persist.tile([P, 2, LQ], BF16)        # [:, pair, :]; head 2p on rows 0:64
            KTt = persist.tile([P, 2, LKV], BF16)
            Vaug = persist.tile([P, KT, HLOCAL, D + 1], BF16)
            attnT = persist.tile([P, 2, LQ], BF16)
            fcw_s = persist.tile([P, 2, E], BF16)
            mT_s = [
                persist.tile([P, LQ], BF16, tag=f"m{kt}", name=f"mT{kt}")
                for kt in range(KT)
            ]
            # zero-padded broadcast operands: row 0 live, rows 1-127 zero so the
            # K=128 outer-product matmul is exact (K<128 matmuls read garbage
            # rows on HW - tile_size rounds up to 32).
            ones64 = persist.tile([P, D], F32)
            rsr_pad = persist.tile([P, LQ], F32)

            # Load the combined ln+exp activation table once: the default
            # greedy table placement picks exp-only / ln-only tables and pays
            # two 1.3us ACT_TABLE_LOADs per softmax normalization.
            nc.scalar.add_instruction(
                mybir.InstLoadActFuncSet(
                    name=nc.scalar.bass.get_next_instruction_name(),
                    act_func_set_id=6,  # natural_log_exp_and_others
                    ins=[],
                    outs=[],
                )
            )
            nc.vector.memset(ones64[:], 0.0)
            nc.vector.memset(ones64[0:1, :], 1.0)
            nc.vector.memset(rsr_pad[:], 0.0)
            nc.gpsimd.memset(Vaug[:, :, :, D:D + 1], 1.0)   # just the ones column

            work = None     # set below; used by emit_scores/emit_norm
            psumSC = None

            def emit_scores(kt, p, h):
                base = h * D
                sc = psumSC.tile([P, LQ], F32, tag="sc", bufs=2,
                                 name=f"sc_{p}{h}{kt}")
                for n in range(NQ):
                    nc.tensor.matmul(
                        sc[:, n * 512:(n + 1) * 512],
                        KTt[base:base + D, p, kt * P:(kt + 1) * P],
                        QT[base:base + D, p, n * 512:(n + 1) * 512],
                        start=True,
                        stop=True,
                    )
                ex = work.tile([P, LQ], BF16, tag="ex", bufs=KT,
                               name=f"ex_{p}{h}{kt}")
                nc.scalar.activation(
                    ex[:], sc[:], mybir.ActivationFunctionType.Exp, scale=0.125
                )
                nc.vector.tensor_tensor(
                    ex[:], ex[:], mT_s[kt][:], mybir.AluOpType.mult
                )
                return ex

            def make_norm(src, p, h, bc_pool):
                # src: [D+1, LQ] accumulator (PSUM or SBUF fp32): rows 0:D are
                # unnormalized att@V, row D the softmax row-sum.
                def emit():
                    base = h * D
                    lnr = work.tile([1, LQ], F32, tag="lnr", bufs=2,
                                    name=f"lnr{p}{h}")
                    nc.scalar.activation(
                        lnr[:], src[D:D + 1, :], mybir.ActivationFunctionType.Ln
                    )
                    nc.scalar.activation(
                        rsr_pad[0:1, :], lnr[:],
                        mybir.ActivationFunctionType.Exp, scale=-1.0,
                    )
                    bc = bc_pool.tile([P, LQ], F32, tag="sc", bufs=2,
                                      name=f"bc{p}{h}")
                    for n in range(NQ):
                        nc.tensor.matmul(
                            bc[0:D, n * 512:(n + 1) * 512],
                            ones64[:],
                            rsr_pad[:, n * 512:(n + 1) * 512],
                            start=True,
                            stop=True,
                        )
                    bcs = work.tile([D, LQ], F32, tag="bcs", bufs=2,
                                    name=f"bcs{p}{h}")
                    nc.vector.tensor_copy(bcs[:], bc[0:D, :])
                    nc.vector.tensor_tensor(
                        attnT[base:base + D, p, :],
                        src[0:D, :],
                        bcs[:],
                        mybir.AluOpType.mult,
                    )
                return emit

            with tc.tile_pool(name="work", bufs=4) as work, \
                 tc.tile_pool(name="psumSC", bufs=1, space="PSUM") as psumSC:
                # ---------------- Phase A + pass-0 scores ----------------
                with (
                    tc.tile_pool(name="inp", bufs=1) as inp,
                    tc.tile_pool(name="psumA", bufs=1, space="PSUM") as psumA,
                ):
                    wq_s = inp.tile([P, ET, FP], BF16, name="wq_s")
                    wk_s = inp.tile([P, ET, FP], BF16, name="wk_s")
                    wv_s = inp.tile([P, ET, FP], BF16, name="wv_s")
                    xT_s = [inp.tile([P, LQ], BF16, tag=f"xT{k}", name=f"xT{k}") for k in range(ET)]
                    cT_s = [inp.tile([P, LKV], BF16, tag=f"cT{k}", name=f"cT{k}") for k in range(ET)]
                    # DMA issue costs ~0.6us per dma_start on the issuing
                    # engine, and HBM bandwidth is ~358GB/s: issue Q operands
                    # first (split across engines), defer wv/masks/fcw until
                    # the first Q tile lands so they don't steal bandwidth.
                    nc.gpsimd.dma_start(
                        wq_s[:, 0:4, :],
                        wqT[0:512, :].rearrange("(ko pi) f -> pi ko f", pi=P),
                    )
                    nc.gpsimd.dma_start(
                        wq_s[:, 4:8, :],
                        wqT[512:1024, :].rearrange("(ko pi) f -> pi ko f", pi=P),
                    )
                    for k in range(4):
                        nc.sync.dma_start(xT_s[k][:], xT[k * P:(k + 1) * P, :])
                    for k in range(4, ET):
                        nc.scalar.dma_start(xT_s[k][:], xT[k * P:(k + 1) * P, :])
                    for k in range(4, ET):
                        nc.gpsimd.dma_start(cT_s[k][:], ctxT[k * P:(k + 1) * P, :])
                    nc.scalar.dma_start(
                        wk_s[:], wkT.rearrange("(ko pi) f -> pi ko f", pi=P)
                    )
                    for k in range(4):
                        nc.scalar.dma_start(cT_s[k][:], ctxT[k * P:(k + 1) * P, :])

                    # Q^T [FP, LQ] in pair-major tiles
                    gate = inp.tile([1, 1], BF16, name="gate")
                    for p in range(2):
                        for n in range(NQ):
                            ps = psumA.tile([P, 512], F32, tag="ps512", bufs=2)
                            for k in range(ET):
                                nc.tensor.matmul(
                                    ps[:],
                                    wq_s[:, k, p * P:(p + 1) * P],
                                    xT_s[k][:, n * 512:(n + 1) * 512],
                                    start=(k == 0),
                                    stop=(k == ET - 1),
                                )
                            nc.vector.tensor_copy(QT[:, p, n * 512:(n + 1) * 512], ps[:])
                            if p == 0 and n == 0:
                                # gate the remaining (later-needed) input DMAs
                                # behind the first Q tile so their transfers
                                # don't compete with x/ctx for HBM bandwidth
                                nc.gpsimd.tensor_copy(gate[:], QT[0:1, 0, 0:1])
                                nc.gpsimd.dma_start(
                                    wv_s[:],
                                    wvT.rearrange("(ko pi) f -> pi ko f", pi=P),
                                )
                                for kt in range(KT):
                                    nc.gpsimd.dma_start(
                                        mT_s[kt][:], maskT[kt * P:(kt + 1) * P, :]
                                    )
                                nc.gpsimd.dma_start(
                                    fcw_s[:],
                                    fcwT.rearrange("(ko pi) e -> pi ko e", pi=P),
                                )

                    # K^T [FP, LKV]
                    for p in range(2):
                        for n in range(LKV // 512):
                            ps = psumA.tile([P, 512], F32, tag="ps512", bufs=2)
                            for k in range(ET):
                                nc.tensor.matmul(
                                    ps[:],
                                    wk_s[:, k, p * P:(p + 1) * P],
                                    cT_s[k][:, n * 512:(n + 1) * 512],
                                    start=(k == 0),
                                    stop=(k == ET - 1),
                                )
                            nc.vector.tensor_copy(KTt[:, p, n * 512:(n + 1) * 512], ps[:])

                    # V natural [LKV, FP] into the ones-augmented tile,
                    # interleaved with pass-0 score tiles (hides pass-0's ACT
                    # exp stream under the V matmuls).
                    ex0 = []
                    for mv in range(KT):
                        ps = psumA.tile([P, FP], F32, tag="psv", bufs=2)
                        for k in range(ET):
                            nc.tensor.matmul(
                                ps[:],
                                cT_s[k][:, mv * P:(mv + 1) * P],
                                wv_s[:, k, :],
                                start=(k == 0),
                                stop=(k == ET - 1),
                            )
                        nc.vector.tensor_copy(
                            Vaug[:, mv, :, 0:D],
                            ps.rearrange("p (h d) -> p h d", d=D),
                        )
                        ex0.append(emit_scores(mv, 0, 0))

                # ---------------- Phase B: attention, one head per pass ----------------
                norm_pending = None
                av3_s = None
                with tc.tile_pool(name="psumAV", bufs=1, space="PSUM") as psumAV:
                    for pi in range(4):
                        p, h = divmod(pi, 2)
                        hh = 2 * p + h
                        av = psumAV.tile([D + 1, LQ], F32, tag="av", bufs=2,
                                         name=f"av{hh}")

                        def attv(okt, oex, av=av, hh=hh):
                            for n in range(NQ):
                                nc.tensor.matmul(
                                    av[:, n * 512:(n + 1) * 512],
                                    Vaug[:, okt, hh, :],
                                    oex[:, n * 512:(n + 1) * 512],
                                    start=(okt == 0),
                                    stop=(okt == KT - 1),
                                )

                        if pi == 0:
                            for kt in range(KT):
                                attv(kt, ex0[kt])
                        else:
                            pend = []
                            for kt in range(KT):
                                pend.append((kt, emit_scores(kt, p, h)))
                                if kt == 3 and norm_pending is not None:
                                    norm_pending()
                                    norm_pending = None
                                if len(pend) > PIPE:
                                    attv(*pend.pop(0))
                            for item in pend:
                                attv(*item)

                        if pi < 3:
                            norm_pending = make_norm(av, p, h, psumSC)
                        else:
                            # evacuate so psumAV can close; normalized under C
                            av3_s = work.tile([D + 1, LQ], F32, tag="av3s",
                                              bufs=1)
                            nc.vector.tensor_copy(av3_s[:], av[:])

                # ---------------- Phase C: output projection ----------------
                # 16 [128,512] chunks; kf0 runs 4 chunks ahead (4 PSUM banks;
                # psumSC still holds the other 4 for pass-3's norm broadcast).
                with (
                    tc.tile_pool(name="psumC", bufs=1, space="PSUM") as psumC,
                    tc.tile_pool(name="outp", bufs=4) as outp,
                ):
                    CCH = 16
                    ps_c = [None] * CCH

                    def fc_mm(c, kf):
                        m, n = divmod(c, NQ)
                        if kf == 0:
                            ps_c[c] = psumC.tile([P, 512], F32, tag="fc", bufs=4,
                                                 name=f"fc{c}")
                        nc.tensor.matmul(
                            ps_c[c][:],
                            fcw_s[:, kf, m * P:(m + 1) * P],
                            attnT[:, kf, n * 512:(n + 1) * 512],
                            start=(kf == 0),
                            stop=(kf == 1),
                        )

                    for c in range(4):
                        fc_mm(c, 0)
                    make_norm(av3_s, 1, 1, psumSC)()
                    for c in range(CCH):
                        fc_mm(c, 1)
                        ob = outp.tile([P, 512], BF16, tag="ob", bufs=4,
                                       name=f"ob{c}")
                        if c % 2 == 0:
                            nc.scalar.copy(ob[:], ps_c[c][:])
                        else:
                            nc.vector.tensor_copy(ob[:], ps_c[c][:])
                        m, n = divmod(c, NQ)
                        nc.sync.dma_start(
                            yT[m * P:(m + 1) * P, n * 512:(n + 1) * 512], ob[:]
                        )
                        if c + 4 < CCH:
                            fc_mm(c + 4, 0)

    nc.compile()
    return nc


_NC_CACHE: dict = {}


def _get_nc() -> bass.Bass:
    if "nc" not in _NC_CACHE:
        _NC_CACHE["nc"] = build_nc()
    return _NC_CACHE["nc"]


def make_in_maps(x, context, pad_mask, Wq, Wk, Wv, fc_w):
    x = np.asarray(x, dtype=np.float32)
    context = np.asarray(context, dtype=np.float32)
    pad_mask = np.asarray(pad_mask).astype(bool)
    Wq = np.asarray(Wq, dtype=np.float32)
    Wk = np.asarray(Wk, dtype=np.float32)
    Wv = np.asarray(Wv, dtype=np.float32)
    fc_w = np.asarray(fc_w, dtype=np.float32)

    xT = np.ascontiguousarray(x.transpose(0, 2, 1)).astype(ml_dtypes.bfloat16)
    cT = np.ascontiguousarray(context.transpose(0, 2, 1)).astype(ml_dtypes.bfloat16)
    keepT = np.ascontiguousarray(
        (~pad_mask).transpose(0, 2, 1)
    ).astype(ml_dtypes.bfloat16)                                    # [B, LKV, LQ]

    in_maps = []
    for c in range(NCORES):
        b, hg = divmod(c, HGROUPS)
        fsl = slice(hg * FP, (hg + 1) * FP)
        in_maps.append(
            {
                "xT": xT[b],
                "ctxT": cT[b],
                "maskT": keepT[b],
                "wqT": np.ascontiguousarray(Wq[fsl, :].T).astype(ml_dtypes.bfloat16),
                "wkT": np.ascontiguousarray(Wk[fsl, :].T).astype(ml_dtypes.bfloat16),
                "wvT": np.ascontiguousarray(Wv[fsl, :].T).astype(ml_dtypes.bfloat16),
                "fcwT": np.ascontiguousarray(fc_w[:, fsl].T).astype(ml_dtypes.bfloat16),
            }
        )
    return in_maps


def _combine(outs, fc_b):
    fc_b = np.asarray(fc_b, dtype=np.float32)
    y = np.empty((B, LQ, E), dtype=np.float32)
    for b in range(B):
        acc = outs[HGROUPS * b].astype(np.float32)
        for g in range(1, HGROUPS):
            acc = acc + outs[HGROUPS * b + g].astype(np.float32)
        y[b] = acc.T + fc_b
    return y


def run_traced(x, context, pad_mask, Wq, Wk, Wv, fc_w, fc_b, trace=False):
    nc = _get_nc()
    in_maps = make_in_maps(x, context, pad_mask, Wq, Wk, Wv, fc_w)
    res = run_bass_kernel_spmd(nc, in_maps, list(range(NCORES)), trace=trace)
    outs = [r["yT"] for r in res.results]
    return _combine(outs, fc_b), res


def kernel(x, context, pad_mask, Wq, Wk, Wv, fc_w, fc_b):
    y, _ = run_traced(x, context, pad_mask, Wq, Wk, Wv, fc_w, fc_b, trace=False)
    return y

